# revision 14
# baseline (speedup 1.0000x reference)
"""AdaptiveTrajectoryDecoder TRN2 Bass kernel (8 NeuronCores, pure data parallel).

Model (per sample, P=12 steps, H=256, E=64, D=2):
    emb   = relu(pos @ We.T + be)                     [E]
    gates = emb @ Wih.T + bih + h @ Whh.T + bhh       [4H]  (torch order i,f,g,o)
    c     = sig(f)*c + sig(i)*tanh(g);  h = sig(o)*tanh(c)
    pred  = pos + h @ Wpos.T + bpos;    pos = pred
    sp    = softplus(relu(h @ Wsp1.T + bsp1) @ Wsp2.T + bsp2)
    un    = exp(relu(h @ Wun1.T + bun1) @ Wun2.T + bun2)

Kernel layout: feature-major ([feature, batch] on chip). Batch 65536 is sharded
8 ways (8192/core), processed as 16 tiles of 512 columns. All matmuls run in
bf16 (f32 PSUM accumulate); c/h and the elementwise chain are bf16; pred/pos
and the sp/un pre-activations stay f32. sigmoid/tanh/relu run in the main loop
(one ACT table set); exp/softplus(=exp then ln(1+x)) run in a small tail pass
after a single table switch, on pre-activations staged through DRAM.

Gate weights are column-permuted host-side to (i, f, o, g) so each PSUM "duo"
([128, 1024] = 2 banks) holds one gate type and gets exactly one activation op.
b_ih+b_hh is folded into a constant-1 extra emb row (K=65 matmul); the other
biases ride the per-partition bias operand of relu / tensor_scalar ops.
"""

import numpy as np
import ml_dtypes

import concourse.bass as bass
import concourse.tile as tile
from concourse import mybir
from concourse.bass_utils import run_bass_kernel_spmd

BF = ml_dtypes.bfloat16
F32 = mybir.dt.float32
BF16 = mybir.dt.bfloat16
AF = mybir.ActivationFunctionType
ALU = mybir.AluOpType

N_CORES = 8
B, H, E, D, P = 65536, 256, 64, 2, 12
BC = B // N_CORES          # 8192 batch per core
NT = 512                   # batch tile (columns)
NJ = BC // NT              # 16 tiles per core
# permuted gate feature-block order: original blocks [i0 i1 f0 f1 g0 g1 o0 o1]
# -> [i0 i1 f0 f1 o0 o1 g0 g1] so duos are (i, f, o, g)
GATE_BLOCK_PERM = [0, 1, 2, 3, 6, 7, 4, 5]

_MAXW = 1  # max sem-waits this walrus build tolerates per instruction


def _patched_drain_and_barrier(self, tick_clock, wait_clock):
    """TileContext exit drain carries one wait per live semaphore; this walrus
    build rejects >2 waits on a Drain. Split them onto SP NoOps instead."""
    nc = self.nc
    probe = nc.sync.nop()
    wait_clock.add_sem_waits(probe.ins, tile.ScopedClock({None: tick_clock.global_clock}))
    si = probe.ins.sync_info
    waits = list(si.on_wait or []) if si else []
    probe.ins.sync_info = mybir.SyncInfo(on_wait=waits[:1], on_update=[])
    for i in range(1, len(waits)):
        extra = nc.sync.nop()
        extra.ins.sync_info = mybir.SyncInfo(on_wait=waits[i:i + 1], on_update=[])
    nc.sync.drain()
    nc.all_engine_barrier()
    assert self.sems is not None
    popped = nc._tile_sem_poison_stack.pop()
    assert popped is self._sem_poison
    nc.clear_and_free_semaphores(list(self.sems.allocated().values()))
    nc.all_engine_barrier()


tile.TileContext._drain_and_barrier = _patched_drain_and_barrier


def _split_excess_waits(nc, maxw=_MAXW):
    """Move excess sem-waits from any instruction onto same-engine NoOps
    inserted immediately before it (per-engine order preserved)."""
    n = 0
    for fn in nc.m.functions:
        for bb in fn.blocks:
            new_insts = []
            for inst in bb.instructions:
                si = getattr(inst, "sync_info", None)
                waits = list(si.on_wait) if si and si.on_wait else []
                if len(waits) > maxw:
                    chunks = [waits[i:i + maxw] for i in range(0, len(waits), maxw)]
                    for chunk in chunks[:-1]:
                        nop = mybir.InstNoOp(
                            name=f"waitsplit-{n}", ins=[], outs=[],
                            engine=inst.engine,
                            sync_info=mybir.SyncInfo(on_wait=chunk, on_update=[]),
                        )
                        n += 1
                        nc.register_instruction(nop, overwrite=True)
                        new_insts.append(nop)
                    inst.sync_info = mybir.SyncInfo(
                        on_wait=chunks[-1], on_update=list(si.on_update or []))
                new_insts.append(inst)
            bb.instructions[:] = new_insts


def build_nc():
    nc = bass.Bass()
    JG = min(4, NJ)            # batch tiles per supergroup (weight reuse factor)
    NSG = NJ // JG             # supergroups per step
    W = JG * NT                # supergroup width in batch columns

    # ---- DRAM parameters (per-core shard shapes) ----
    h0 = nc.declare_dram_parameter("h0", [H, BC], BF16, isOutput=False)
    c0 = nc.declare_dram_parameter("c0", [H, BC], BF16, isOutput=False)
    pos0 = nc.declare_dram_parameter("pos0", [D, BC], F32, isOutput=False)
    wg = nc.declare_dram_parameter("wg", [H, 4 * H], BF16, isOutput=False)      # Whh.T col-perm
    wi65 = nc.declare_dram_parameter("wi65", [E + 1, 4 * H], BF16, isOutput=False)  # [Wih.T; bih+bhh] col-perm
    wspun1 = nc.declare_dram_parameter("wspun1", [H, 128], BF16, isOutput=False)    # [Wsp1;Wun1].T
    bspun1 = nc.declare_dram_parameter("bspun1", [128, 1], F32, isOutput=False)
    wpos5 = nc.declare_dram_parameter("wpos5", [H, 35], BF16, isOutput=False)   # cols 0:2 = Wpos.T
    wsu5 = nc.declare_dram_parameter("wsu5", [128, 35], BF16, isOutput=False)   # col 32 = Wsp2.T, 33:35 = Wun2.T
    b5 = nc.declare_dram_parameter("b5", [35, 1], F32, isOutput=False)          # rows 0:2 bpos, 32 bsp2, 33:35 bun2
    wemb = nc.declare_dram_parameter("wemb", [D, E], BF16, isOutput=False)      # We.T
    bemb = nc.declare_dram_parameter("bemb", [E, 1], F32, isOutput=False)

    preds_out = nc.declare_dram_parameter("preds_out", [P, D, BC], F32, isOutput=True)
    speeds_out = nc.declare_dram_parameter("speeds_out", [P, BC], F32, isOutput=True)
    un_out = nc.declare_dram_parameter("un_out", [P, D, BC], F32, isOutput=True)

    with tile.TileContext(nc) as tc:
        with (
            tc.tile_pool(name="persist", bufs=1) as pp,
            tc.tile_pool(name="work", bufs=1) as wp,
            tc.tile_pool(name="ps", bufs=2, space="PSUM") as ps,
            tc.tile_pool(name="dstage", bufs=1, space="DRAM") as dp,
        ):
            # ---- persistent SBUF state (h/c are half-major: [k_half][j][b]) ----
            h_sb = pp.tile([128, 2 * BC], BF16)
            c_sb = pp.tile([128, 2 * BC], BF16)
            pos_sb = pp.tile([D, BC], F32)
            emb_sb = pp.tile([E + 1, BC], BF16)         # row E is constant 1.0

            wg_sb = pp.tile([128, 2, 4 * H], BF16)      # [k_half][gate feature col]
            wi_sb = pp.tile([E + 1, 4 * H], BF16)
            wspun1_sb = pp.tile([128, 2, 128], BF16)
            bspun1_sb = pp.tile([128, 1], F32)
            wpos5_sb = pp.tile([128, 2, 35], BF16)
            wsu5_sb = pp.tile([128, 35], BF16)
            b5_sb = pp.tile([35, 1], F32)
            wemb_sb = pp.tile([D, E], BF16)
            bemb_sb = pp.tile([E, 1], F32)

            stage = dp.tile([P, 3, BC], BF16)           # rows: sp_pre, un_pre0/1

            # ---- loads (h/c half-major -> contiguous DMAs) ----
            for k in range(2):
                nc.sync.dma_start(h_sb[:, k * BC:(k + 1) * BC],
                                  h0[k * 128:(k + 1) * 128, :])
                nc.sync.dma_start(c_sb[:, k * BC:(k + 1) * BC],
                                  c0[k * 128:(k + 1) * 128, :])
                nc.sync.dma_start(wg_sb[:, k, :], wg[k * 128:(k + 1) * 128, :])
                nc.sync.dma_start(wspun1_sb[:, k, :], wspun1[k * 128:(k + 1) * 128, :])
                nc.sync.dma_start(wpos5_sb[:, k, :], wpos5[k * 128:(k + 1) * 128, :])
            nc.sync.dma_start(pos_sb[:], pos0[:])
            nc.sync.dma_start(wi_sb[:], wi65[:])
            nc.sync.dma_start(bspun1_sb[:], bspun1[:])
            nc.sync.dma_start(wsu5_sb[:], wsu5[:])
            nc.sync.dma_start(b5_sb[:], b5[:])
            nc.sync.dma_start(wemb_sb[:], wemb[:])
            nc.sync.dma_start(bemb_sb[:], bemb[:])
            nc.vector.memset(emb_sb[E:E + 1, :], 1.0)

            def h_half(j, k):                       # [128, NT] matmul rhs
                return h_sb[:, k * BC + j * NT:k * BC + (j + 1) * NT]

            def hc_seg(buf, k, sg):                 # [128, W] half-slice of sg
                return buf[:, k * BC + sg * W:k * BC + (sg + 1) * W]

            def emb_sg(sg):
                return emb_sb[:, sg * W:(sg + 1) * W]

            def pos_sg(sg):
                return pos_sb[:, sg * W:(sg + 1) * W]

            def cast_pos(t, sg):
                pb = wp.tile([D, W], BF16, tag="posbf", name=f"pb_{t}_{sg}")
                nc.vector.tensor_copy(pb[:], pos_sg(sg))
                return pb

            def emit_emb(t, sg, pos_bf):
                """emb(t, sg) = relu(pos_bf @ We.T + be); one LDW, JG matmuls."""
                q = ps.tile([128, W], F32, tag="quad", name=f"embq_{t}_{sg}")
                for jj in range(JG):
                    nc.tensor.matmul(q[0:E, jj * NT:(jj + 1) * NT], wemb_sb[:],
                                     pos_bf[:, jj * NT:(jj + 1) * NT],
                                     start=True, stop=True)
                nc.vector.tensor_scalar(
                    out=emb_sg(sg)[0:E, :], in0=q[0:E, :],
                    scalar1=bemb_sb[:, 0:1], scalar2=0.0,
                    op0=ALU.add, op1=ALU.max)

            def emit_gates(t, sg):
                """8 gate m-tile quads (each: 3 weight loads x JG matmuls, one
                activation), then the bf16 c update. h happens in emit_hpart."""
                acts = []
                for m in range(8):
                    func = AF.Tanh if m >= 6 else AF.Sigmoid   # order i,f,o,g
                    q = ps.tile([128, W], F32, tag="quad", name=f"g{m}_{t}_{sg}")
                    for k in range(3):
                        lhsT = (wg_sb[:, k, m * 128:(m + 1) * 128] if k < 2
                                else wi_sb[:, m * 128:(m + 1) * 128])
                        for jj in range(JG):
                            j = sg * JG + jj
                            rhs = (h_half(j, k) if k < 2
                                   else emb_sb[:, j * NT:(j + 1) * NT])
                            nc.tensor.matmul(q[:, jj * NT:(jj + 1) * NT], lhsT, rhs,
                                             start=(k == 0), stop=(k == 2))
                    bufs = 2 if m in (4, 5) else 1   # sig_o survives one group
                    s = wp.tile([128, W], BF16, tag=f"act{m}", bufs=bufs,
                                name=f"s{m}_{t}_{sg}")
                    nc.scalar.activation(s[:], q[:], func)
                    acts.append(s)
                for k in range(2):  # c = sig(f)*c + sig(i)*tanh(g), per K-half
                    ck = hc_seg(c_sb, k, sg)
                    t1 = wp.tile([128, W], BF16, tag=f"t1_{k}", name=f"t1_{k}_{t}_{sg}")
                    nc.vector.tensor_mul(t1[:], acts[2 + k][:], ck)
                    t2 = wp.tile([128, W], BF16, tag=f"t2_{k}", name=f"t2_{k}_{t}_{sg}")
                    nc.vector.tensor_mul(t2[:], acts[0 + k][:], acts[6 + k][:])
                    nc.vector.tensor_add(ck, t1[:], t2[:])
                return (acts[4], acts[5])  # sig_o halves

            def emit_hpart(t, sg, sig_o):
                """h = sig(o) * tanh(c) for supergroup sg (inputs ready)."""
                for k in range(2):
                    ck = hc_seg(c_sb, k, sg)
                    hk = hc_seg(h_sb, k, sg)
                    tc_t = wp.tile([128, W], BF16, tag=f"tanh_c{k}",
                                   name=f"tc{k}_{t}_{sg}")
                    nc.scalar.activation(tc_t[:], ck, AF.Tanh)
                    nc.vector.tensor_mul(hk, sig_o[k][:], tc_t[:])

            def emit_small(t, sg):
                """spun1, (pred|sp2|un2), staging for supergroup sg."""
                q = ps.tile([128, W], F32, tag="quad", name=f"sm1_{t}_{sg}")
                for k in range(2):
                    for jj in range(JG):
                        nc.tensor.matmul(q[:, jj * NT:(jj + 1) * NT],
                                         wspun1_sb[:, k, :], h_half(sg * JG + jj, k),
                                         start=(k == 0), stop=(k == 1))
                spun = wp.tile([128, W], BF16, tag="spun", name=f"spun_{t}_{sg}")
                nc.vector.tensor_scalar(
                    out=spun[:], in0=q[:], scalar1=bspun1_sb[:, 0:1], scalar2=0.0,
                    op0=ALU.add, op1=ALU.max)
                p5 = ps.tile([128, W], F32, tag="quad", name=f"p5_{t}_{sg}")
                for k in range(2):
                    for jj in range(JG):
                        nc.tensor.matmul(p5[0:35, jj * NT:(jj + 1) * NT],
                                         wpos5_sb[:, k, :], h_half(sg * JG + jj, k),
                                         start=(k == 0), stop=False)
                for jj in range(JG):
                    nc.tensor.matmul(p5[0:35, jj * NT:(jj + 1) * NT], wsu5_sb[:],
                                     spun[:, jj * NT:(jj + 1) * NT],
                                     start=False, stop=True)
                # pred = (p5[0:2] + bpos) + pos   (in-place into pos slice)
                nc.vector.scalar_tensor_tensor(
                    out=pos_sg(sg), in0=p5[0:2, :], scalar=b5_sb[0:2, 0:1],
                    in1=pos_sg(sg), op0=ALU.add, op1=ALU.add)
                # sp/un pre-activations -> bf16 SBUF -> DRAM staging
                st = wp.tile([3, W], BF16, tag="stage", bufs=2, name=f"st_{t}_{sg}")
                nc.vector.tensor_scalar(
                    out=st[:], in0=p5[32:35, :], scalar1=b5_sb[32:35, 0:1],
                    scalar2=None, op0=ALU.add)
                nc.sync.dma_start(stage[t, :, sg * W:(sg + 1) * W], st[:])
                nc.sync.dma_start(preds_out[t, :, sg * W:(sg + 1) * W], pos_sg(sg))
                if t + 1 < P:
                    return cast_pos(t, sg)
                return None

            # ---- prologue: emb(0, sg) for all supergroups ----
            for sg in range(NSG):
                emit_emb(0, sg, cast_pos(-1, sg))

            # ---- main loop, small phase software-pipelined by one supergroup ----
            sgs = [(t, sg) for t in range(P) for sg in range(NSG)]
            inline_emb = NSG < 2
            pend = None
            emb_q = []
            for n, (t, sg) in enumerate(sgs):
                while emb_q:
                    emit_emb(*emb_q.pop(0))
                if pend is not None:
                    emit_hpart(*pend)
                so = emit_gates(t, sg)
                if pend is not None:
                    pt, psg = pend[0], pend[1]
                    pb = emit_small(pt, psg)
                    if pb is not None:
                        if inline_emb:
                            emit_emb(pt + 1, psg, pb)
                        else:
                            emb_q.append((pt + 1, psg, pb))
                pend = (t, sg, so)
            emit_hpart(*pend)
            emit_small(pend[0], pend[1])

            # ---- tail: exp / softplus on staged pre-activations ----
            # sp: softplus(x) = ln(1 + exp(x));  un: exp(x)
            ch = BC // 128
            spw = P * ch
            sp_in = wp.tile([128, spw], BF16, tag="act0", bufs=1)
            for t in range(P):
                nc.sync.dma_start(sp_in[:, t * ch:(t + 1) * ch], stage[t, 0, :])
            sp_e = wp.tile([128, spw], F32, tag="act1", bufs=1)
            nc.scalar.activation(sp_e[:], sp_in[:], AF.Exp)
            sp_o = wp.tile([128, spw], F32, tag="act2", bufs=1)
            nc.scalar.activation(sp_o[:], sp_e[:], AF.Ln, bias=1.0)
            for t in range(P):
                nc.sync.dma_start(speeds_out[t, :], sp_o[:, t * ch:(t + 1) * ch])

            un_in = wp.tile([128, 2 * spw], BF16, tag="act3", bufs=1)
            for t in range(P):
                nc.sync.dma_start(un_in[:, 2 * t * ch:2 * (t + 1) * ch],
                                  stage[t, 1:3, :])
            un_o = wp.tile([128, 2 * spw], F32, tag="act6", bufs=1)
            nc.scalar.activation(un_o[:], un_in[:], AF.Exp)
            for t in range(P):
                nc.sync.dma_start(un_out[t, :, :], un_o[:, 2 * t * ch:2 * (t + 1) * ch])

    _split_excess_waits(nc)
    return nc


_NC_CACHE = None


def _get_nc():
    global _NC_CACHE
    if _NC_CACHE is None:
        _NC_CACHE = build_nc()
    return _NC_CACHE


def kernel(encoder_hidden, encoder_cell, last_position,
           W_embed, b_embed, W_ih, W_hh, b_ih, b_hh,
           W_pos, b_pos, W_sp1, b_sp1, W_sp2, b_sp2,
           W_un1, b_un1, W_un2, b_un2):
    f32 = np.float32
    encoder_hidden = np.asarray(encoder_hidden, f32)
    encoder_cell = np.asarray(encoder_cell, f32)
    last_position = np.asarray(last_position, f32)

    perm = np.concatenate([np.arange(128) + 128 * b for b in GATE_BLOCK_PERM])

    wg_np = np.ascontiguousarray(np.asarray(W_hh, f32).T[:, perm]).astype(BF)
    bg = (np.asarray(b_ih, f32) + np.asarray(b_hh, f32))[perm]
    wi65_np = np.ascontiguousarray(
        np.concatenate([np.asarray(W_ih, f32).T[:, perm], bg[None, :]], axis=0)
    ).astype(BF)
    wspun1_np = np.ascontiguousarray(
        np.concatenate([np.asarray(W_sp1, f32), np.asarray(W_un1, f32)], 0).T
    ).astype(BF)
    bspun1_np = np.concatenate(
        [np.asarray(b_sp1, f32), np.asarray(b_un1, f32)])[:, None].astype(f32)
    wpos5_np = np.zeros((H, 35), f32)
    wpos5_np[:, 0:2] = np.asarray(W_pos, f32).T
    wpos5_np = wpos5_np.astype(BF)
    wsu5_np = np.zeros((128, 35), f32)
    wsu5_np[0:64, 32] = np.asarray(W_sp2, f32)[0]
    wsu5_np[64:128, 33:35] = np.asarray(W_un2, f32).T
    wsu5_np = wsu5_np.astype(BF)
    b5_np = np.zeros((35, 1), f32)
    b5_np[0:2, 0] = np.asarray(b_pos, f32)
    b5_np[32, 0] = np.asarray(b_sp2, f32)[0]
    b5_np[33:35, 0] = np.asarray(b_un2, f32)
    wemb_np = np.ascontiguousarray(np.asarray(W_embed, f32).T).astype(BF)
    bemb_np = np.asarray(b_embed, f32)[:, None]

    eh_t = np.ascontiguousarray(encoder_hidden.T).astype(BF)   # [H, B]
    ec_t = np.ascontiguousarray(encoder_cell.T).astype(BF)
    lp_t = np.ascontiguousarray(last_position.T)               # [D, B] f32

    in_maps = []
    for c in range(N_CORES):
        sl = slice(c * BC, (c + 1) * BC)
        in_maps.append({
            "h0": np.ascontiguousarray(eh_t[:, sl]),
            "c0": np.ascontiguousarray(ec_t[:, sl]),
            "pos0": np.ascontiguousarray(lp_t[:, sl]),
            "wg": wg_np, "wi65": wi65_np, "wspun1": wspun1_np,
            "bspun1": bspun1_np, "wpos5": wpos5_np, "wsu5": wsu5_np,
            "b5": b5_np, "wemb": wemb_np, "bemb": bemb_np,
        })

    nc = _get_nc()
    res = run_bass_kernel_spmd(nc, in_maps, core_ids=list(range(N_CORES)))

    preds = np.empty((B, P, D), f32)
    speeds = np.empty((B, P, 1), f32)
    uns = np.empty((B, P, D), f32)
    for c in range(N_CORES):
        sl = slice(c * BC, (c + 1) * BC)
        r = res.results[c]
        preds[sl] = np.transpose(r["preds_out"], (2, 0, 1))
        speeds[sl, :, 0] = r["speeds_out"].T
        uns[sl] = np.transpose(r["un_out"], (2, 0, 1))
    return preds, speeds, uns
